# revision 9
# baseline (speedup 1.0000x reference)
"""Distributed Trainium2 kernel for a 16-head attention layer.

Problem: B=2, L=2048, HID=1024, H=16 (torch-Linear projections, masked softmax).
Sharding: 8 cores = batch (2) x query-chunk (4). The mask zeroes ~half the keys
uniformly across queries/heads (shape [B,1,1,L]), so masked keys are dropped
host-side: keys are compacted to NV valid keys padded to LT_K tiles of 128.

Pad keys are handled without any mask bias: pad K columns are zeroed so their
scores are exactly 0 and exp(0)=1 contributes a host-known constant NPAD to
each softmax denominator, which is subtracted before the reciprocal. Pad V
rows are zero so the numerator is clean (requires bv == 0, asserted host-side;
true for this problem).

Attention layout (contraction dims on SBUF partitions throughout):
  scores are computed transposed (S.T[lk, lq]) so P.T feeds PV directly.
  The softmax denominator is FUSED into the PV matmul: the V stationary for
  head h is [V_h | ones] (65 columns, from the vhx2 [128, 16*65] layout built
  by the V projection), so each per-head PSUM bank [65, LQ] accumulates the
  attention numerator on partitions 0:64 and the denominator on partition 64
  with zero extra PE work. Normalization: reciprocal on partition 64, PE
  rank-1 broadcast to lanes 0:64, DVE multiply; the pair's second head is
  then moved to attnT partitions 64:128 with a cross-partition SBUF DMA.
"""

import math
import sys
import types

import numpy as np
import ml_dtypes

# ---- problem constants (hardcoded; kernel.py must be self-contained) ----
B, L, HID, H = 2, 2048, 1024, 16
DH = HID // H          # 64
N_CORES = 8
GROUP = N_CORES // B   # 4 cores per batch group
LQ = (B * L) // N_CORES  # 512 queries per core
P = 128
KT = HID // P          # 8 contraction tiles
OT = HID // P          # 8 output tiles
NPAIR = H // 2         # 8 head pairs
SCALE = DH ** -0.5
BF16 = ml_dtypes.bfloat16
HW = 65                # per-head stationary width in vhx2: [V_h | ones]
DBG = False            # emit debug intermediate dumps (core-local)


def _ensure_profile_hook():
    """Install the NTFF profiling hook trn_boot couldn't (antenv.axon_hooks
    is missing from the image); harmless if profiling is never requested."""
    if "antenv.axon_hooks" in sys.modules:
        return
    try:
        from trn_agent_boot.trn_boot import _ntff_profile_via_ctypes

        hook = _ntff_profile_via_ctypes("/opt/axon/libaxon_pjrt.so")
    except Exception:
        hook = None
    mod = types.ModuleType("antenv.axon_hooks")
    mod.get_axon_ntff_profile_hook = lambda: hook
    mod.set_axon_ntff_profile_hook = lambda h: None
    sys.modules["antenv.axon_hooks"] = mod


def build_bass(lt_k):
    """Build + compile the per-core Bass program (same graph on all 8 cores).

    lt_k: number of 128-key tiles attention iterates over (compacted+padded).
    """
    import concourse.mybir as mybir
    import concourse.tile as tile
    from concourse import bacc

    f32 = mybir.dt.float32
    bf16 = mybir.dt.bfloat16
    ADD = mybir.AluOpType.add
    MULT = mybir.AluOpType.mult
    EXP = mybir.ActivationFunctionType.Exp

    LK = lt_k * P          # attention key width

    nc = bacc.Bacc("TRN2", target_bir_lowering=False, debug=False, num_devices=N_CORES)

    qT = nc.declare_dram_parameter("qT", [HID, LQ], bf16, isOutput=False)
    kTl = nc.declare_dram_parameter("kTl", [HID, LK], bf16, isOutput=False)
    vTl = nc.declare_dram_parameter("vTl", [HID, LK], bf16, isOutput=False)
    WqT = nc.declare_dram_parameter("WqT", [HID, HID], bf16, isOutput=False)
    WkT = nc.declare_dram_parameter("WkT", [HID, HID], bf16, isOutput=False)
    WvT = nc.declare_dram_parameter("WvT", [HID, HID], bf16, isOutput=False)
    WoT = nc.declare_dram_parameter("WoT", [HID, HID], bf16, isOutput=False)
    bq = nc.declare_dram_parameter("bq", [P, OT], f32, isOutput=False)
    bk = nc.declare_dram_parameter("bk", [P, OT], f32, isOutput=False)
    bo = nc.declare_dram_parameter("bo", [P, OT], f32, isOutput=False)
    bv_row = nc.declare_dram_parameter("bv_row", [1, HID], bf16, isOutput=False)
    cpn = nc.declare_dram_parameter("cpn", [P, 1], f32, isOutput=False)
    out = nc.declare_dram_parameter("out", [HID, LQ], f32, isOutput=True)
    if DBG:
        d_vhx2 = nc.declare_dram_parameter("d_vhx2", [P, H * HW], f32, isOutput=True)
        d_pt = nc.declare_dram_parameter("d_pt", [P, 1024], f32, isOutput=True)
        d_pv = nc.declare_dram_parameter("d_pv", [HW, LQ], f32, isOutput=True)
        d_rcb = nc.declare_dram_parameter("d_rcb", [1, 1024], f32, isOutput=True)
        d_rbs = nc.declare_dram_parameter("d_rbs", [DH, 1024], f32, isOutput=True)
        d_at = nc.declare_dram_parameter("d_at", [P, LQ], f32, isOutput=True)

    with tile.TileContext(nc) as tc:
        with (
            tc.tile_pool(name="consts", bufs=1) as consts,
            tc.tile_pool(name="khT", bufs=OT) as khT_p,
            tc.tile_pool(name="vhx2", bufs=lt_k) as vhx2_p,
            tc.tile_pool(name="qhT", bufs=OT) as qhT_p,
            tc.tile_pool(name="attnT", bufs=NPAIR) as attnT_p,
            tc.tile_pool(name="pt", bufs=10) as pt_p,
            tc.tile_pool(name="nrm", bufs=6) as nrm_p,
            tc.tile_pool(name="wq", bufs=KT) as wq_p,
            tc.tile_pool(name="qTin", bufs=KT) as qT_p,
            tc.tile_pool(name="wo", bufs=KT) as wo_p,
            tc.tile_pool(name="osb", bufs=2) as osb_p,
            tc.tile_pool(name="psum", bufs=1, space="PSUM") as psum,
        ):
            # ---- constants (DMAs deferred until after the first K-proj
            # weight tiles so they don't delay the first matmul) ----
            ones_row = consts.tile([1, P], bf16, tag="ones_row")
            nc.vector.memset(ones_row[:], 1.0)
            # ones on lane 64, for the per-head recip row broadcasts
            ones_rb = consts.tile([P, DH], bf16, tag="ones_rb")
            nc.vector.memset(ones_rb[:], 1.0)

            # ---- K projection: khT[ot] [128, LK] ----
            khT = []
            with (
                tc.tile_pool(name="wk", bufs=KT) as wk_p,
                tc.tile_pool(name="kin", bufs=KT) as kin_p,
            ):
                wk_sb = []
                kin_sb = []
                for i in range(KT):
                    w = wk_p.tile([P, HID], bf16, tag="wk")
                    nc.sync.dma_start(w[:], WkT[i * P : (i + 1) * P, :])
                    wk_sb.append(w)
                    x = kin_p.tile([P, LK], bf16, tag="kin")
                    nc.sync.dma_start(x[:], kTl[i * P : (i + 1) * P, :])
                    kin_sb.append(x)
                    if i == 0:
                        # bk feeds the first K-proj bias — keep it right
                        # behind the first weight tile on the DMA queue
                        bk_sb = consts.tile([P, OT], f32, tag="bk")
                        nc.sync.dma_start(bk_sb[:], bk[:])
                cpn_sb = consts.tile([P, 1], f32, tag="cpn")
                nc.sync.dma_start(cpn_sb[:], cpn[:])
                bq_sb = consts.tile([P, OT], f32, tag="bq")
                nc.sync.dma_start(bq_sb[:], bq[:])
                bo_sb = consts.tile([P, OT], f32, tag="bo")
                nc.sync.dma_start(bo_sb[:], bo[:])
                bv_sb = consts.tile([1, HID], bf16, tag="bvr")
                nc.sync.dma_start(bv_sb[:], bv_row[:])
                for ot in range(OT):
                    t = khT_p.tile([P, LK], bf16, tag="khT", name=f"khT{ot}")
                    for c0 in range(0, LK, 512):
                        cw = min(512, LK - c0)
                        ps = psum.tile([P, 1024], f32, tag="mm", bufs=2)
                        for i in range(KT):
                            nc.tensor.matmul(
                                ps[:, 0:cw],
                                wk_sb[i][:, ot * P : (ot + 1) * P],
                                kin_sb[i][:, c0 : c0 + cw],
                                start=(i == 0),
                                stop=(i == KT - 1),
                            )
                        nc.vector.tensor_scalar(
                            t[:, c0 : c0 + cw],
                            ps[:, 0:cw],
                            bk_sb[:, ot : ot + 1],
                            None,
                            op0=ADD,
                        )
                    khT.append(t)
                # v-bias broadcast tile [128, HID] via rank-1 ones matmul
                # (emitted after K-proj so it fills the K->V transition)
                bvb_ps = psum.tile([P, 1024], f32, tag="mm", bufs=2)
                for h2 in range(2):
                    nc.tensor.matmul(
                        bvb_ps[:, h2 * 512 : (h2 + 1) * 512],
                        ones_row[:, :],
                        bv_sb[:, h2 * 512 : (h2 + 1) * 512],
                        start=True,
                        stop=True,
                    )
                bvb = consts.tile([P, HID], f32, tag="bvb")
                nc.vector.tensor_copy(bvb[:], bvb_ps[:])

            # ---- V projection into vhx2[j] [128 keys, 16*65]: per head the
            # 64 projected columns followed by a ones column (the fused
            # denominator stationary) ----
            vhx2 = []
            bvb3 = bvb[:].rearrange("p (h w) -> p h w", w=DH)
            with (
                tc.tile_pool(name="wv", bufs=KT) as wv_p,
                tc.tile_pool(name="vin", bufs=KT) as vin_p,
            ):
                wv_sb = []
                vin_sb = []
                for i in range(KT):
                    w = wv_p.tile([P, HID], bf16, tag="wv")
                    nc.sync.dma_start(w[:], WvT[i * P : (i + 1) * P, :])
                    wv_sb.append(w)
                    x = vin_p.tile([P, LK], bf16, tag="vin")
                    nc.sync.dma_start(x[:], vTl[i * P : (i + 1) * P, :])
                    vin_sb.append(x)
                for jl in range(lt_k):
                    ps = psum.tile([P, 1024], f32, tag="mm", bufs=2)
                    for half in range(2):
                        for i in range(KT):
                            nc.tensor.matmul(
                                ps[:, half * 512 : (half + 1) * 512],
                                vin_sb[i][:, jl * P : (jl + 1) * P],
                                wv_sb[i][:, half * 512 : (half + 1) * 512],
                                start=(i == 0),
                                stop=(i == KT - 1),
                            )
                    t = vhx2_p.tile([P, H * HW], bf16, tag="vhx2",
                                    name=f"vhx2{jl}")
                    v3 = t[:].rearrange("p (h w) -> p h w", w=HW)
                    nc.vector.memset(v3[:, :, DH:HW], 1.0)
                    nc.vector.tensor_tensor(
                        v3[:, :, 0:DH],
                        ps[:].rearrange("p (h w) -> p h w", w=DH),
                        bvb3,
                        op=ADD,
                    )
                    if DBG and jl == 0:
                        dvt = consts.tile([P, H * HW], f32, tag="dvt")
                        nc.vector.tensor_copy(dvt[:], t[:])
                        nc.sync.dma_start(d_vhx2[:], dvt[:])
                    vhx2.append(t)

            # ---- Q projection: qhT[ot] [128, LQ] bf16. Only the first pair's
            # tiles are emitted up front; the rest interleave with attention
            # groups to fill the PE slack while ScalarE works through the exps.
            qhT = []
            wq_sb = []
            qT_sb = []
            for i in range(KT):
                w = wq_p.tile([P, HID], bf16, tag="wq")
                nc.sync.dma_start(w[:], WqT[i * P : (i + 1) * P, :])
                wq_sb.append(w)
                x = qT_p.tile([P, LQ], bf16, tag="qTin")
                nc.sync.dma_start(x[:], qT[i * P : (i + 1) * P, :])
                qT_sb.append(x)

            def emit_qproj(ot):
                ps = psum.tile([P, 1024], f32, tag="mm", bufs=2,
                               name=f"qps{ot}")
                for i in range(KT):
                    nc.tensor.matmul(
                        ps[:, 0:LQ],
                        wq_sb[i][:, ot * P : (ot + 1) * P],
                        qT_sb[i][:, :],
                        start=(i == 0),
                        stop=(i == KT - 1),
                    )
                t = qhT_p.tile([P, LQ], bf16, tag="qhT", name=f"qhT{ot}")
                nc.vector.tensor_scalar(
                    t[:], ps[:, 0:LQ], bq_sb[:, ot : ot + 1], None, op0=ADD
                )
                qhT.append(t)

            emit_qproj(0)
            emit_qproj(1)

            # output-projection weights, loaded ahead so the tail can overlap
            wo_sb = []
            for i in range(KT):
                w = wo_p.tile([P, HID], bf16, tag="wo")
                nc.sync.dma_start(w[:], WoT[i * P : (i + 1) * P, :])
                wo_sb.append(w)
            o_ps = {}

            def emit_oproj_mms(ot, i_lo, i_hi):
                if ot not in o_ps:
                    o_ps[ot] = psum.tile([P, 1024], f32, tag="mm", bufs=2,
                                         name=f"ops{ot}")
                for i in range(i_lo, i_hi):
                    nc.tensor.matmul(
                        o_ps[ot][:, 0:LQ],
                        wo_sb[i][:, ot * P : (ot + 1) * P],
                        attnT[i][:, :],
                        start=(i == 0),
                        stop=(i == KT - 1),
                    )

            def emit_oproj_out(ot):
                o = osb_p.tile([P, LQ], f32, tag="osb", name=f"osb{ot}")
                nc.vector.tensor_scalar(
                    o[:], o_ps[ot][:, 0:LQ], bo_sb[:, ot : ot + 1], None, op0=ADD
                )
                nc.sync.dma_start(out[ot * P : (ot + 1) * P, :], o[:])

            # ---- attention: pairs in groups of 2, pipelined over j ----
            attnT = []
            pts = {}
            for g in range(NPAIR // 2):
                pairs = (2 * g, 2 * g + 1)
                # per-head PSUM banks [65, LQ]: numerator 0:64, denom at 64
                pv = {}
                for hp in pairs:
                    for h in (2 * hp, 2 * hp + 1):
                        pv[h] = psum.tile([HW, LQ], f32, tag="pv", bufs=4,
                                          name=f"pv{h}")
                # software-pipelined j-loop: QK+exp for step j are emitted with
                # PV for step j-1, so the PE queue never waits on the exp
                # that produces the pt it is about to consume.
                def emit_qk_exp(hp, j):
                    sp = psum.tile([P, 1024], f32, tag="mm", bufs=2,
                                   name=f"sp{hp}_{j}")
                    nc.tensor.matmul(
                        sp[:, 0:512],
                        khT[hp][0:DH, j * P : (j + 1) * P],
                        qhT[hp][0:DH, :],
                        start=True,
                        stop=True,
                    )
                    nc.tensor.matmul(
                        sp[:, 512:1024],
                        khT[hp][DH:P, j * P : (j + 1) * P],
                        qhT[hp][DH:P, :],
                        start=True,
                        stop=True,
                    )
                    pt = pt_p.tile([P, 1024], bf16, tag="pt",
                                   name=f"pt{hp}_{j}")
                    nc.scalar.activation(pt[:], sp[:], EXP, bias=0.0,
                                         scale=SCALE)
                    if DBG and hp == 0 and j == 0:
                        dpt = consts.tile([P, 1024], f32, tag="dpt")
                        nc.vector.tensor_copy(dpt[:], pt[:])
                        nc.sync.dma_start(d_pt[:], dpt[:])
                    pts[(hp, j)] = pt

                def emit_pv(hp, j):
                    pt = pts.pop((hp, j))
                    for half in range(2):
                        h = 2 * hp + half
                        nc.tensor.matmul(
                            pv[h][0:HW, :],
                            vhx2[j][:, h * HW : (h + 1) * HW],
                            pt[:, half * 512 : (half + 1) * 512],
                            start=(j == 0),
                            stop=(j == lt_k - 1),
                        )

                for j in range(lt_k):
                    for hp in pairs:
                        if (hp, j) not in pts:
                            emit_qk_exp(hp, j)
                    if j > 0:
                        for hp in pairs:
                            emit_pv(hp, j - 1)
                # fill the wait on the final exps: emit the next pairs' Q
                # projection and pre-issue the next group's first QK+exp
                if g < NPAIR // 2 - 1:
                    emit_qproj(2 * g + 2)
                    emit_qproj(2 * g + 3)
                    for hp in (2 * g + 2, 2 * g + 3):
                        emit_qk_exp(hp, 0)
                for hp in pairs:
                    emit_pv(hp, lt_k - 1)
                # ---- normalize each pair: denominators live on partition 64
                # of the per-head pv banks ----
                for hp in pairs:
                    hA, hB = 2 * hp, 2 * hp + 1
                    if DBG and hp == 0:
                        dpv = consts.tile([HW, LQ], f32, tag="dpv")
                        nc.vector.tensor_copy(dpv[0:HW, :], pv[0][0:HW, :])
                        nc.sync.dma_start(d_pv[:], dpv[:])
                    rcf = nrm_p.tile([P, 1024], f32, tag="rcf", bufs=2,
                                     name=f"rcf{hp}")
                    nc.vector.tensor_scalar(
                        rcf[DH:DH + 1, 0:512], pv[hA][DH:DH + 1, :],
                        cpn_sb[DH:DH + 1, 0:1], None, op0=ADD,
                    )
                    nc.vector.tensor_scalar(
                        rcf[DH:DH + 1, 512:1024], pv[hB][DH:DH + 1, :],
                        cpn_sb[DH:DH + 1, 0:1], None, op0=ADD,
                    )
                    rcg = nrm_p.tile([P, 1024], f32, tag="rcg", bufs=2,
                                     name=f"rcg{hp}")
                    # exact reciprocal: approx_fast is a custom-DVE op that
                    # misbehaves at non-zero partition base, and this is only
                    # [1, 1024] per pair
                    nc.vector.reciprocal(
                        rcg[DH:DH + 1, :], rcf[DH:DH + 1, :]
                    )
                    rcb = nrm_p.tile([P, 1024], bf16, tag="rcb", bufs=2,
                                     name=f"rcb{hp}")
                    nc.vector.tensor_copy(rcb[DH:DH + 1, :], rcg[DH:DH + 1, :])
                    # PE rank-1 broadcast: recip rows (lane 64) -> lanes 0:64
                    rbp = psum.tile([P, 1024], f32, tag="mm", bufs=2,
                                    name=f"rbp{hp}")
                    for half in range(2):
                        nc.tensor.matmul(
                            rbp[0:DH, half * 512 : (half + 1) * 512],
                            ones_rb[DH : DH + 1, :],
                            rcb[DH : DH + 1, half * 512 : (half + 1) * 512],
                            start=True,
                            stop=True,
                            tile_position=(DH, 0),
                        )
                    rbs = nrm_p.tile([DH, 1024], bf16, tag="rbs", bufs=2,
                                     name=f"rbs{hp}")
                    nc.vector.tensor_copy(rbs[0:DH, :], rbp[0:DH, :])
                    if DBG and hp == 0:
                        drc = consts.tile([P, 1024], f32, tag="drc")
                        nc.vector.tensor_copy(drc[DH:DH + 1, :], rcb[DH:DH + 1, :])
                        nc.sync.dma_start(d_rcb[:], drc[DH:DH + 1, :])
                        drb = consts.tile([DH, 1024], f32, tag="drb")
                        nc.vector.tensor_copy(drb[0:DH, :], rbs[0:DH, :])
                        nc.sync.dma_start(d_rbs[:], drb[:])
                    at = attnT_p.tile([P, LQ], bf16, tag="attnT",
                                      name=f"at{hp}")
                    nc.vector.tensor_tensor(
                        at[0:DH, :], pv[hA][0:DH, :], rbs[0:DH, 0:512], op=MULT
                    )
                    tmpb = nrm_p.tile([DH, LQ], bf16, tag="tmpb", bufs=2,
                                      name=f"tmpb{hp}")
                    nc.vector.tensor_tensor(
                        tmpb[0:DH, :], pv[hB][0:DH, :], rbs[0:DH, 512:1024],
                        op=MULT,
                    )
                    # cross-partition move: head B to attnT partitions 64:128
                    # (two DMAs so two queues split the latency)
                    nc.sync.dma_start(at[DH:DH + 32, :], tmpb[0:32, :])
                    nc.sync.dma_start(at[DH + 32 : P, :], tmpb[32:DH, :])
                    if DBG and hp == 0:
                        dat = consts.tile([P, LQ], f32, tag="dat")
                        nc.vector.tensor_copy(dat[:], at[:])
                        nc.sync.dma_start(d_at[:], dat[:])
                    attnT.append(at)

            # ---- output projection (emitted after the last normalize so the
            # "mm" psum rotation never wedges against the rbp broadcasts; the
            # PE still overlaps the final normalize chain since attnT[0..5]
            # are already resident) ----
            for ot in range(OT):
                emit_oproj_mms(ot, 0, KT)
                emit_oproj_out(ot)
                del o_ps[ot]

    nc.compile()
    return nc


def _plan(mask):
    """Compaction plan from the mask: valid key indices per batch + tiling."""
    mask = np.asarray(mask)
    idxs = [np.where(mask[b, 0, 0, :] != 0)[0] for b in range(B)]
    nv = max((len(ix) for ix in idxs), default=1)
    nv = max(nv, 1)
    lt_k = max(1, math.ceil(nv / P))
    return idxs, lt_k


def make_in_maps(q, k, v, mask, Wq, bq, Wk, bk, Wv, bv, Wo, bo, idxs, lt_k):
    """Shard + lay out the full inputs for the 8 cores (host-side numpy)."""
    q = np.asarray(q, np.float32)
    k = np.asarray(k, np.float32)
    v = np.asarray(v, np.float32)
    assert np.abs(np.asarray(bv)).max() == 0.0, (
        "pad-key handling assumes bv == 0 (pad V rows must be exactly zero)"
    )

    LK = lt_k * P

    def t_bf16(a):  # [R, C] -> contiguous [C, R] bf16
        return np.ascontiguousarray(np.asarray(a, np.float32).T).astype(BF16)

    WqT_h, WkT_h, WvT_h, WoT_h = (t_bf16(w) for w in (Wq, Wk, Wv, Wo))

    def b_tiles(b):  # [HID] -> [128, 8] f32 (per-o-tile partition vectors)
        return np.ascontiguousarray(np.asarray(b, np.float32).reshape(OT, P).T)

    bq_h, bk_h, bo_h = b_tiles(bq), b_tiles(bk), b_tiles(bo)
    bv_h = np.asarray(bv, np.float32)[None, :].astype(BF16)

    per_batch = {}
    for b in range(B):
        ix = idxs[b]
        nvb = len(ix)
        kc = np.zeros((LK, HID), np.float32)
        vc = np.zeros((LK, HID), np.float32)
        kc[:nvb] = k[b][ix]
        vc[:nvb] = v[b][ix]
        kcT = t_bf16(kc)  # [HID, LK]
        vcT = t_bf16(vc)
        cpn_h = np.full((P, 1), -(LK - nvb), np.float32)
        per_batch[b] = (kcT, vcT, cpn_h)

    in_maps = []
    for c in range(N_CORES):
        b, ch = divmod(c, GROUP)
        r0 = ch * LQ
        kcT, vcT, cpn_h = per_batch[b]
        in_maps.append(
            {
                "qT": t_bf16(q[b, r0 : r0 + LQ, :]),
                "kTl": kcT,
                "vTl": vcT,
                "WqT": WqT_h,
                "WkT": WkT_h,
                "WvT": WvT_h,
                "WoT": WoT_h,
                "bq": bq_h,
                "bk": bk_h,
                "bo": bo_h,
                "bv_row": bv_h,
                "cpn": cpn_h,
            }
        )
    return in_maps


def assemble_output(results):
    """Gather per-core out.T [HID, LQ] slices into the full [B, L, HID]."""
    full = np.empty((B, L, HID), np.float32)
    for c in range(N_CORES):
        b, ch = divmod(c, GROUP)
        r0 = ch * LQ
        full[b, r0 : r0 + LQ, :] = results[c]["out"].T
    return full


_NC_CACHE = {}


def _run(trace=False, **inputs):
    _ensure_profile_hook()
    from concourse.bass_utils import run_bass_kernel_spmd
    from concourse import bass_utils

    bass_utils.upload_artifacts = lambda tmpdir: tmpdir  # zero-egress container
    idxs, lt_k = _plan(inputs["mask"])
    print(f"plan: nv={[len(ix) for ix in idxs]} lt_k={lt_k}", flush=True)
    if lt_k not in _NC_CACHE:
        _NC_CACHE[lt_k] = build_bass(lt_k)
    in_maps = make_in_maps(
        **{k: v for k, v in inputs.items()}, idxs=idxs, lt_k=lt_k
    )
    res = run_bass_kernel_spmd(
        _NC_CACHE[lt_k], in_maps, core_ids=list(range(N_CORES)), trace=trace
    )
    return assemble_output(res.results), res


def kernel(**inputs):
    out, _ = _run(trace=False, **inputs)
    return out


# revision 27
# speedup vs baseline: 1.4185x; 1.4185x over previous
"""Distributed Trainium2 kernel for a 16-head attention layer.

Problem: B=2, L=2048, HID=1024, H=16 (torch-Linear projections, masked softmax).
Sharding: 8 cores = batch (2) x query-chunk (4). The mask zeroes ~half the keys
uniformly across queries/heads (shape [B,1,1,L]), so masked keys are dropped
host-side: keys are compacted to NV valid keys padded to LT_K tiles of 128.

Pad keys are handled without any mask bias: pad K columns are zeroed so their
scores are exactly 0 and exp(0)=1 contributes a host-known constant NPAD to
each softmax denominator, which is subtracted before the reciprocal. Pad V
rows are zero so the numerator is clean (requires bv == 0, asserted host-side;
true for this problem).

Attention layout (contraction dims on SBUF partitions throughout):
  scores are computed transposed (S.T[lk, lq]) so P.T feeds PV directly.
  The softmax denominator is FUSED into the PV matmul: the V stationary for
  head h is [V_h | ones] (65 columns, from the vhx2 [128, 16*65] layout built
  by the V projection), so each per-head PSUM bank [65, LQ] accumulates the
  attention numerator on partitions 0:64 and the denominator on partition 64
  with zero extra PE work. Normalization: reciprocal on partition 64, PE
  rank-1 broadcast to lanes 0:64, DVE multiply; the pair's second head is
  then moved to attnT partitions 64:128 with a cross-partition SBUF DMA.
"""

import math
import sys
import types

import numpy as np
import ml_dtypes

# ---- problem constants (hardcoded; kernel.py must be self-contained) ----
B, L, HID, H = 2, 2048, 1024, 16
DH = HID // H          # 64
N_CORES = 8
GROUP = N_CORES // B   # 4 cores per batch group
LQ = (B * L) // N_CORES  # 512 queries per core
P = 128
KT = HID // P          # 8 contraction tiles
OT = HID // P          # 8 output tiles
NPAIR = H // 2         # 8 head pairs
SCALE = DH ** -0.5
BF16 = ml_dtypes.bfloat16
HW = 65                # per-head stationary width in vhx2: [V_h | ones]
DBG = False            # emit debug intermediate dumps (core-local)


def _ensure_profile_hook():
    """Install the NTFF profiling hook trn_boot couldn't (antenv.axon_hooks
    is missing from the image); harmless if profiling is never requested."""
    if "antenv.axon_hooks" in sys.modules:
        return
    try:
        from trn_agent_boot.trn_boot import _ntff_profile_via_ctypes

        hook = _ntff_profile_via_ctypes("/opt/axon/libaxon_pjrt.so")
    except Exception:
        hook = None
    mod = types.ModuleType("antenv.axon_hooks")
    mod.get_axon_ntff_profile_hook = lambda: hook
    mod.set_axon_ntff_profile_hook = lambda h: None
    sys.modules["antenv.axon_hooks"] = mod


def build_bass(lt_k):
    """Build + compile the per-core Bass program (same graph on all 8 cores).

    lt_k: number of 128-key tiles attention iterates over (compacted+padded).

    Schedule: the attention j-loop is ScalarE-exp paced (940ns per pair-step
    vs 853ns of PE work), so all remaining projection work (K-proj ot2..7,
    Q-proj ot2..7, O-proj partial sums) is emitted INSIDE the j-loops as
    evenly spread filler units — the PE stays saturated, holds its max
    p-state, and the exp stream hides completely.
    """
    import concourse.mybir as mybir
    import concourse.tile as tile
    from concourse import bacc

    f32 = mybir.dt.float32
    bf16 = mybir.dt.bfloat16
    ADD = mybir.AluOpType.add
    MULT = mybir.AluOpType.mult
    EXP = mybir.ActivationFunctionType.Exp

    LK = lt_k * P          # attention key width
    NCH = (LK + 511) // 512  # 512-col chunks per K-proj output tile

    nc = bacc.Bacc("TRN2", target_bir_lowering=False, debug=False, num_devices=N_CORES)

    qT = nc.declare_dram_parameter("qT", [HID, LQ], bf16, isOutput=False)
    kTl = nc.declare_dram_parameter("kTl", [HID, LK], bf16, isOutput=False)
    vTl = nc.declare_dram_parameter("vTl", [HID, LK], bf16, isOutput=False)
    WqT = nc.declare_dram_parameter("WqT", [HID, HID], bf16, isOutput=False)
    WkT = nc.declare_dram_parameter("WkT", [HID, HID], bf16, isOutput=False)
    WvT = nc.declare_dram_parameter("WvT", [HID, HID], bf16, isOutput=False)
    WoT = nc.declare_dram_parameter("WoT", [HID, HID], bf16, isOutput=False)
    bq = nc.declare_dram_parameter("bq", [P, OT], f32, isOutput=False)
    bk = nc.declare_dram_parameter("bk", [P, OT], f32, isOutput=False)
    bo = nc.declare_dram_parameter("bo", [P, OT], f32, isOutput=False)
    bv_row = nc.declare_dram_parameter("bv_row", [1, HID], bf16, isOutput=False)
    cpn = nc.declare_dram_parameter("cpn", [P, 4], f32, isOutput=False)
    out = nc.declare_dram_parameter("out", [HID, LQ], f32, isOutput=True)
    if DBG:
        d_vhx2 = nc.declare_dram_parameter("d_vhx2", [P, H * HW], f32, isOutput=True)
        d_pt = nc.declare_dram_parameter("d_pt", [P, 1024], f32, isOutput=True)
        d_pv = nc.declare_dram_parameter("d_pv", [HW, LQ], f32, isOutput=True)
        d_rcb = nc.declare_dram_parameter("d_rcb", [1, 1024], f32, isOutput=True)
        d_rbs = nc.declare_dram_parameter("d_rbs", [DH, 1024], f32, isOutput=True)
        d_at = nc.declare_dram_parameter("d_at", [P, LQ], f32, isOutput=True)

    with tile.TileContext(nc) as tc:
        with (
            tc.tile_pool(name="consts", bufs=1) as consts,
            tc.tile_pool(name="wk", bufs=KT) as wk_p,
            tc.tile_pool(name="kin", bufs=KT) as kin_p,
            tc.tile_pool(name="khT", bufs=OT) as khT_p,
            tc.tile_pool(name="vhx2", bufs=lt_k) as vhx2_p,
            tc.tile_pool(name="qhT", bufs=OT) as qhT_p,
            tc.tile_pool(name="attnT", bufs=NPAIR) as attnT_p,
            tc.tile_pool(name="pt", bufs=10) as pt_p,
            tc.tile_pool(name="wq", bufs=KT) as wq_p,
            tc.tile_pool(name="qTin", bufs=KT) as qT_p,
            tc.tile_pool(name="wo", bufs=KT) as wo_p,
            tc.tile_pool(name="psum", bufs=1, space="PSUM") as psum,
        ):
            # ---- constants ----
            ones_row = consts.tile([1, P], bf16, tag="ones_row")
            nc.vector.memset(ones_row[:], 1.0)
            # ones on lane 64, for the per-head recip row broadcasts
            ones_rb = consts.tile([P, DH], bf16, tag="ones_rb")
            nc.vector.memset(ones_rb[:], 1.0)
            bk_sb = consts.tile([P, OT], f32, tag="bk")
            nc.sync.dma_start(bk_sb[:], bk[:])
            cpn_sb = consts.tile([P, 4], f32, tag="cpn")
            nc.sync.dma_start(cpn_sb[:], cpn[:])
            bq_sb = consts.tile([P, OT], f32, tag="bq")
            nc.sync.dma_start(bq_sb[:], bq[:])
            bo_sb = consts.tile([P, OT], f32, tag="bo")
            nc.sync.dma_start(bo_sb[:], bo[:])
            bv_sb = consts.tile([1, HID], bf16, tag="bvr")
            nc.sync.dma_start(bv_sb[:], bv_row[:])

            # v-bias broadcast tile [128, HID] via rank-1 ones matmul
            bvb_ps = psum.tile([P, 1024], f32, tag="sp", bufs=2)
            for h2 in range(2):
                nc.tensor.matmul(
                    bvb_ps[:, h2 * 512 : (h2 + 1) * 512],
                    ones_row[:, :],
                    bv_sb[:, h2 * 512 : (h2 + 1) * 512],
                    start=True,
                    stop=True,
                )
            bvb = consts.tile([P, HID], f32, tag="bvb")
            nc.vector.tensor_copy(bvb[:], bvb_ps[:])

            # ---- V projection into vhx2[j] [128 keys, 16*65]: per head the
            # 64 projected columns followed by a ones column (the fused
            # denominator stationary) ----
            vhx2 = []
            bvb3 = bvb[:].rearrange("p (h w) -> p h w", w=DH)
            with (
                tc.tile_pool(name="wv", bufs=KT) as wv_p,
                tc.tile_pool(name="vin0", bufs=KT) as vin0_p,
                tc.tile_pool(name="vin", bufs=KT) as vin_p,
            ):
                # the first key-tile's input columns ship as separate small
                # DMAs so V-proj jl=0 starts after ~2.25MB instead of 4MB
                wv_sb = []
                vin0_sb = []
                vin_sb = []
                for i in range(KT):
                    w = wv_p.tile([P, HID], bf16, tag="wv")
                    nc.sync.dma_start(w[:], WvT[i * P : (i + 1) * P, :])
                    wv_sb.append(w)
                    x0 = vin0_p.tile([P, P], bf16, tag="vin0")
                    nc.sync.dma_start(x0[:], vTl[i * P : (i + 1) * P, 0:P])
                    vin0_sb.append(x0)
                for i in range(KT):
                    x = vin_p.tile([P, LK - P], bf16, tag="vin")
                    nc.sync.dma_start(x[:], vTl[i * P : (i + 1) * P, P:LK])
                    vin_sb.append(x)

                def vin_ap(i, jl):
                    if jl == 0:
                        return vin0_sb[i][:, 0:P]
                    return vin_sb[i][:, (jl - 1) * P : jl * P]

                for jl in range(lt_k):
                    ps = psum.tile([P, 1024], f32, tag="sp", bufs=2)
                    for half in range(2):
                        for i in range(KT):
                            nc.tensor.matmul(
                                ps[:, half * 512 : (half + 1) * 512],
                                vin_ap(i, jl),
                                wv_sb[i][:, half * 512 : (half + 1) * 512],
                                start=(i == 0),
                                stop=(i == KT - 1),
                            )
                    t = vhx2_p.tile([P, H * HW], bf16, tag="vhx2",
                                    name=f"vhx2{jl}")
                    v3 = t[:].rearrange("p (h w) -> p h w", w=HW)
                    nc.vector.memset(v3[:, :, DH:HW], 1.0)
                    nc.vector.tensor_tensor(
                        v3[:, :, 0:DH],
                        ps[:].rearrange("p (h w) -> p h w", w=DH),
                        bvb3,
                        op=ADD,
                    )
                    if DBG and jl == 0:
                        dvt = consts.tile([P, H * HW], f32, tag="dvt")
                        nc.vector.tensor_copy(dvt[:], t[:])
                        nc.sync.dma_start(d_vhx2[:], dvt[:])
                    vhx2.append(t)

            # ---- K projection inputs (all 8 contract tiles stay resident:
            # kproj for ot 2..7 runs inside the attention loop) ----
            wk_sb = []
            kin_sb = []
            for i in range(KT):
                w = wk_p.tile([P, HID], bf16, tag="wk")
                nc.sync.dma_start(w[:], WkT[i * P : (i + 1) * P, :])
                wk_sb.append(w)
                x = kin_p.tile([P, LK], bf16, tag="kin")
                nc.sync.dma_start(x[:], kTl[i * P : (i + 1) * P, :])
                kin_sb.append(x)


            kh_tiles = [None] * OT

            def emit_kproj_chunk(ot, c0, pre=False):
                if kh_tiles[ot] is None:
                    kh_tiles[ot] = khT_p.tile([P, LK], bf16, tag="khT",
                                              name=f"khT{ot}")
                t = kh_tiles[ot]
                cw = min(512, LK - c0)
                # pre-phase chunks double-buffer on the idle "sp" tag; the
                # single "fill" bank is for units inside the attention loop
                if pre:
                    ps = psum.tile([P, 1024], f32, tag="sp", bufs=2,
                                   name=f"kps{ot}_{c0}")
                else:
                    ps = psum.tile([P, 512], f32, tag="fill", bufs=1,
                                   name=f"kps{ot}_{c0}")
                for i in range(KT):
                    nc.tensor.matmul(
                        ps[:, 0:cw],
                        wk_sb[i][:, ot * P : (ot + 1) * P],
                        kin_sb[i][:, c0 : c0 + cw],
                        start=(i == 0),
                        stop=(i == KT - 1),
                    )
                nc.vector.tensor_scalar(
                    t[:, c0 : c0 + cw], ps[:, 0:cw], bk_sb[:, ot : ot + 1],
                    None, op0=ADD,
                )


            # pre-phase K-proj: only the two tiles attention needs first
            for ot in (0, 1):
                for c0 in range(0, LK, 512):
                    emit_kproj_chunk(ot, c0, pre=True)

            # ---- Q projection inputs + the first two output tiles ----
            qhT = [None] * OT
            wq_sb = []
            qT_sb = []
            for i in range(KT):
                w = wq_p.tile([P, HID], bf16, tag="wq")
                nc.sync.dma_start(w[:], WqT[i * P : (i + 1) * P, :])
                wq_sb.append(w)
                x = qT_p.tile([P, LQ], bf16, tag="qTin")
                nc.sync.dma_start(x[:], qT[i * P : (i + 1) * P, :])
                qT_sb.append(x)

            def emit_qproj(ot, pre=False):
                if pre:
                    ps = psum.tile([P, 1024], f32, tag="sp", bufs=2,
                                   name=f"qps{ot}")
                else:
                    ps = psum.tile([P, 512], f32, tag="fill", bufs=1,
                                   name=f"qps{ot}")
                for i in range(KT):
                    nc.tensor.matmul(
                        ps[:, 0:LQ],
                        wq_sb[i][:, ot * P : (ot + 1) * P],
                        qT_sb[i][:, :],
                        start=(i == 0),
                        stop=(i == KT - 1),
                    )
                t = qhT_p.tile([P, LQ], bf16, tag="qhT", name=f"qhT{ot}")
                nc.vector.tensor_scalar(
                    t[:], ps[:, 0:LQ], bq_sb[:, ot : ot + 1], None, op0=ADD
                )
                qhT[ot] = t

            emit_qproj(0, pre=True)
            emit_qproj(1, pre=True)

            # output-projection weights, loaded ahead so the tail can overlap
            wo_sb = []
            for i in range(KT):
                w = wo_p.tile([P, HID], bf16, tag="wo")
                nc.sync.dma_start(w[:], WoT[i * P : (i + 1) * P, :])
                wo_sb.append(w)

            # ---- attention: pairs in groups of 2, pipelined over j, with
            # normalize-of-previous-group and projection filler units spread
            # across the j-steps ----
            with tc.tile_pool(name="nrm", bufs=1) as nrm_p:
                attnT = [None] * NPAIR
                pts = {}
                pvs = {}
                opart = {}

                def emit_qk_exp(hp, j):
                    sp = psum.tile([P, 1024], f32, tag="sp", bufs=2,
                                   name=f"sp{hp}_{j}")
                    nc.tensor.matmul(
                        sp[:, 0:512],
                        kh_tiles[hp][0:DH, j * P : (j + 1) * P],
                        qhT[hp][0:DH, :],
                        start=True,
                        stop=True,
                    )
                    nc.tensor.matmul(
                        sp[:, 512:1024],
                        kh_tiles[hp][DH:P, j * P : (j + 1) * P],
                        qhT[hp][DH:P, :],
                        start=True,
                        stop=True,
                    )
                    pt = pt_p.tile([P, 1024], bf16, tag="pt",
                                   name=f"pt{hp}_{j}")
                    nc.scalar.activation(pt[:], sp[:], EXP, bias=0.0,
                                         scale=SCALE)
                    if DBG and hp == 0 and j == 0:
                        dpt = consts.tile([P, 1024], f32, tag="dpt")
                        nc.vector.tensor_copy(dpt[:], pt[:])
                        nc.sync.dma_start(d_pt[:], dpt[:])
                    pts[(hp, j)] = pt

                def emit_pv(hp, j):
                    pt = pts.pop((hp, j))
                    for half in range(2):
                        h = 2 * hp + half
                        nc.tensor.matmul(
                            pvs[h][0:HW, :],
                            vhx2[j][:, h * HW : (h + 1) * HW],
                            pt[:, half * 512 : (half + 1) * 512],
                            start=(j == 0),
                            stop=(j == lt_k - 1),
                        )

                def normalize_pair(hp):
                    """Reciprocal + scale for the pair's two heads.

                    Denominators sit on partition 64 (single lane) of the
                    per-head pv banks; single-lane DVE ops cost N cycles, so
                    the chain is: lane-64 cast+pad-subtract to bf16, PE
                    rank-1 broadcast of the RAW denominator to lanes 0:64,
                    then reciprocal_approx_fast on 64 lanes at partition
                    base 0 (the custom op misbehaves at non-zero base),
                    then the DVE multiplies.
                    """
                    ln = slice(DH, DH + 1)
                    hA, hB = 2 * hp, 2 * hp + 1
                    if DBG and hp == 0:
                        dpv = consts.tile([HW, LQ], f32, tag="dpv")
                        nc.vector.tensor_copy(dpv[0:HW, :], pvs[0][0:HW, :])
                        nc.sync.dma_start(d_pv[:], dpv[:])
                    IDN = mybir.ActivationFunctionType.Identity
                    rcb = nrm_p.tile([P, 1024], bf16, tag="rcb", bufs=2,
                                     name=f"rcb{hp}")
                    # lane-64 cast+pad-subtract on ScalarE (keeps the chain
                    # off the loaded DVE queue; ScalarE has slack here)
                    nc.scalar.activation(rcb[ln, 0:512], pvs[hA][ln, :],
                                         IDN, bias=cpn_sb[ln, 0:1])
                    nc.scalar.activation(rcb[ln, 512:1024], pvs[hB][ln, :],
                                         IDN, bias=cpn_sb[ln, 0:1])
                    # PE rank-1 broadcast: dn rows -> lanes 0:64
                    rbs = nrm_p.tile([DH, 1024], f32, tag="rbs", bufs=2,
                                     name=f"rbs{hp}")
                    rbp = psum.tile([P, 1024], f32, tag="sp", bufs=2,
                                    name=f"rbp{hp}")
                    for half in range(2):
                        nc.tensor.matmul(
                            rbp[0:DH, half * 512 : (half + 1) * 512],
                            ones_rb[ln, :],
                            rcb[ln, half * 512 : (half + 1) * 512],
                            start=True,
                            stop=True,
                            tile_position=(DH, 0),
                        )
                    nc.vector.reciprocal_approx_fast(rbs[0:DH, :],
                                                     rbp[0:DH, :])
                    at = attnT_p.tile([P, LQ], bf16, tag="attnT",
                                      name=f"at{hp}")
                    nc.vector.tensor_tensor(
                        at[0:DH, :], pvs[hA][0:DH, :],
                        rbs[0:DH, 0:512], op=MULT,
                    )
                    tmpb = nrm_p.tile([DH, LQ], bf16, tag="tmpb", bufs=2,
                                      name=f"tmpb{hp}")
                    nc.vector.tensor_tensor(
                        tmpb[0:DH, :], pvs[hB][0:DH, :],
                        rbs[0:DH, 512:1024], op=MULT,
                    )
                    # cross-partition move: head B -> attnT lanes 64:128
                    # (4 DMAs so 4 queues split the ~64KB latency)
                    for q4 in range(4):
                        nc.sync.dma_start(
                            at[DH + 16 * q4 : DH + 16 * (q4 + 1), :],
                            tmpb[16 * q4 : 16 * (q4 + 1), :],
                        )
                    if DBG and hp == 0:
                        drc = consts.tile([P, 1024], f32, tag="drc")
                        nc.vector.tensor_copy(drc[ln, :], rcb[ln, :])
                        nc.sync.dma_start(d_rcb[:], drc[ln, :])
                        drb = consts.tile([DH, 1024], f32, tag="drb")
                        nc.vector.tensor_copy(drb[0:DH, :], rbs[0:DH, :])
                        nc.sync.dma_start(d_rbs[:], drb[:])
                        dat = consts.tile([P, LQ], f32, tag="dat")
                        nc.vector.tensor_copy(dat[:], at[:])
                        nc.sync.dma_start(d_at[:], dat[:])
                    attnT[hp] = at
                    del pvs[hA], pvs[hB]

                def emit_oproj_partial(ot, i_hi=6):
                    ps = psum.tile([P, 512], f32, tag="fill", bufs=1,
                                   name=f"opp{ot}")
                    for i in range(i_hi):
                        nc.tensor.matmul(
                            ps[:, 0:LQ],
                            wo_sb[i][:, ot * P : (ot + 1) * P],
                            attnT[i][:, :],
                            start=(i == 0),
                            stop=(i == i_hi - 1),
                        )
                    t = nrm_p.tile([P, LQ], f32, tag="opart", bufs=OT,
                                   name=f"opart{ot}")
                    nc.vector.tensor_scalar(
                        t[:], ps[:, 0:LQ], bo_sb[:, ot : ot + 1], None,
                        op0=ADD,
                    )
                    opart[ot] = (t, i_hi)

                for gp in range(NPAIR):
                    hp = gp
                    for h in (2 * hp, 2 * hp + 1):
                        pvs[h] = psum.tile([HW, LQ], f32, tag="pv",
                                           bufs=3, name=f"pv{h}")
                    # filler units: pair gp produces khT/qhT for pair gp+2;
                    # pairs 6,7 instead run the o-proj partial sums
                    units = []
                    if gp < NPAIR - 2:
                        ot = gp + 2
                        for c0 in range(0, LK, 512):
                            units.append(
                                lambda ot=ot, c0=c0: emit_kproj_chunk(ot, c0))
                        units.append(lambda ot=ot: emit_qproj(ot))
                    elif gp == NPAIR - 2:
                        for ot in range(4):
                            units.append(
                                lambda ot=ot: emit_oproj_partial(ot))
                    else:
                        for ot in (4, 5):
                            units.append(
                                lambda ot=ot: emit_oproj_partial(ot, 7))
                    j_lo = 1
                    nsteps = max(lt_k - j_lo, 1)
                    done = 0
                    for j in range(lt_k):
                        if (hp, j) not in pts:
                            emit_qk_exp(hp, j)
                        if j > 0:
                            emit_pv(hp, j - 1)
                        if j >= j_lo:
                            target = (len(units) * (j - j_lo + 1)
                                      + nsteps - 1) // nsteps
                            while done < min(target, len(units)):
                                units[done]()
                                done += 1
                    while done < len(units):
                        units[done]()
                        done += 1
                    emit_pv(hp, lt_k - 1)
                    if gp < NPAIR - 1:
                        # pre-issue the next pair's first QK+exp: PE cover
                        # while this pair's normalize chain runs
                        emit_qk_exp(hp + 1, 0)
                    if gp == NPAIR - 1:
                        # staggered tail: the last two o-proj partials run
                        # while the final pair's chain drains on the DVE
                        emit_oproj_partial(6, 7)
                        emit_oproj_partial(7, 7)
                    normalize_pair(hp)

                # ---- output projection tail: contract steps 6,7 + the
                # partial-sum add, software-pipelined so the PE's next unit
                # runs while the previous unit's DVE add drains
                o2s = {}

                def emit_o2_mms(ot):
                    i_lo = opart[ot][1]
                    if ot % 2 == 0:
                        o2s[ot] = psum.tile([P, 512], f32, tag="fill",
                                            bufs=1, name=f"o2{ot}")
                    else:
                        o2s[ot] = psum.tile([P, 1024], f32, tag="sp",
                                            bufs=2, name=f"o2{ot}")
                    for i in range(i_lo, KT):
                        nc.tensor.matmul(
                            o2s[ot][:, 0:LQ],
                            wo_sb[i][:, ot * P : (ot + 1) * P],
                            attnT[i][:, :],
                            start=(i == i_lo),
                            stop=(i == KT - 1),
                        )

                def emit_o2_add(ot):
                    o = nrm_p.tile([P, LQ], f32, tag="osb", bufs=4,
                                   name=f"osb{ot}")
                    nc.vector.tensor_tensor(
                        o[:], o2s.pop(ot)[:, 0:LQ], opart[ot][0][:], op=ADD
                    )
                    nc.sync.dma_start(out[ot * P : ot * P + DH, :],
                                      o[0:DH, :])
                    nc.sync.dma_start(out[ot * P + DH : (ot + 1) * P, :],
                                      o[DH:P, :])

                for ot in range(OT):
                    emit_o2_mms(ot)
                    if ot > 0:
                        emit_o2_add(ot - 1)
                emit_o2_add(OT - 1)

    nc.compile()
    return nc


def _plan(mask):
    """Compaction plan from the mask: valid key indices per batch + tiling."""
    mask = np.asarray(mask)
    idxs = [np.where(mask[b, 0, 0, :] != 0)[0] for b in range(B)]
    nv = max((len(ix) for ix in idxs), default=1)
    nv = max(nv, 1)
    lt_k = max(1, math.ceil(nv / P))
    return idxs, lt_k


def make_in_maps(q, k, v, mask, Wq, bq, Wk, bk, Wv, bv, Wo, bo, idxs, lt_k):
    """Shard + lay out the full inputs for the 8 cores (host-side numpy)."""
    q = np.asarray(q, np.float32)
    k = np.asarray(k, np.float32)
    v = np.asarray(v, np.float32)
    assert np.abs(np.asarray(bv)).max() == 0.0, (
        "pad-key handling assumes bv == 0 (pad V rows must be exactly zero)"
    )

    LK = lt_k * P

    def t_bf16(a):  # [R, C] -> contiguous [C, R] bf16
        return np.ascontiguousarray(np.asarray(a, np.float32).T).astype(BF16)

    WqT_h, WkT_h, WvT_h, WoT_h = (t_bf16(w) for w in (Wq, Wk, Wv, Wo))

    def b_tiles(b):  # [HID] -> [128, 8] f32 (per-o-tile partition vectors)
        return np.ascontiguousarray(np.asarray(b, np.float32).reshape(OT, P).T)

    bq_h, bk_h, bo_h = b_tiles(bq), b_tiles(bk), b_tiles(bo)
    bv_h = np.asarray(bv, np.float32)[None, :].astype(BF16)

    per_batch = {}
    for b in range(B):
        ix = idxs[b]
        nvb = len(ix)
        kc = np.zeros((LK, HID), np.float32)
        vc = np.zeros((LK, HID), np.float32)
        kc[:nvb] = k[b][ix]
        vc[:nvb] = v[b][ix]
        kcT = t_bf16(kc)  # [HID, LK]
        vcT = t_bf16(vc)
        y0 = 1.0 / (max(nvb, 1) * math.exp(0.5))
        cpn_h = np.zeros((P, 4), np.float32)
        cpn_h[:, 0] = -(LK - nvb)
        cpn_h[:, 1] = -y0
        cpn_h[:, 2] = y0
        per_batch[b] = (kcT, vcT, cpn_h)

    in_maps = []
    for c in range(N_CORES):
        b, ch = divmod(c, GROUP)
        r0 = ch * LQ
        kcT, vcT, cpn_h = per_batch[b]
        in_maps.append(
            {
                "qT": t_bf16(q[b, r0 : r0 + LQ, :]),
                "kTl": kcT,
                "vTl": vcT,
                "WqT": WqT_h,
                "WkT": WkT_h,
                "WvT": WvT_h,
                "WoT": WoT_h,
                "bq": bq_h,
                "bk": bk_h,
                "bo": bo_h,
                "bv_row": bv_h,
                "cpn": cpn_h,
            }
        )
    return in_maps


def assemble_output(results):
    """Gather per-core out.T [HID, LQ] slices into the full [B, L, HID]."""
    full = np.empty((B, L, HID), np.float32)
    for c in range(N_CORES):
        b, ch = divmod(c, GROUP)
        r0 = ch * LQ
        full[b, r0 : r0 + LQ, :] = results[c]["out"].T
    return full


_NC_CACHE = {}


def _run(trace=False, **inputs):
    _ensure_profile_hook()
    from concourse.bass_utils import run_bass_kernel_spmd
    from concourse import bass_utils

    bass_utils.upload_artifacts = lambda tmpdir: tmpdir  # zero-egress container
    idxs, lt_k = _plan(inputs["mask"])
    print(f"plan: nv={[len(ix) for ix in idxs]} lt_k={lt_k}", flush=True)
    if lt_k not in _NC_CACHE:
        _NC_CACHE[lt_k] = build_bass(lt_k)
    in_maps = make_in_maps(
        **{k: v for k, v in inputs.items()}, idxs=idxs, lt_k=lt_k
    )
    res = run_bass_kernel_spmd(
        _NC_CACHE[lt_k], in_maps, core_ids=list(range(N_CORES)), trace=trace
    )
    return assemble_output(res.results), res


def kernel(**inputs):
    out, _ = _run(trace=False, **inputs)
    return out
